# revision 27
# baseline (speedup 1.0000x reference)
"""Trainium2 Bass kernel for implicit cross-attention (keys/values = queries + 1 ctx token).

Sharding: 8 cores = 4 batches x 2 head-groups (8 heads each). Each core computes
q = x_b @ Wq[:, g], causal attention over keys [ctx, q_0..q_{N-1}], and a
partial output projection out @ Wo[g, :]. Host sums the two head-group partials
per batch and adds the bias.

Schedule: queries processed in two 1024-column chunks. Per chunk: projection
(PE transposes + matmuls), ctx-score rows (batched exp), v-block build, then
attention per head with a software-pipelined score->exp->U skew so the
ScalarE exp stream stays off the PE critical path. The output projection of
chunk 0 is emitted after attention of chunk 1 so it fills PE gaps while
ScalarE drains exps; chunk 1's projection likewise overlaps attention of
chunk 0 via the Tile list scheduler.
"""

import numpy as np

import concourse.bass as bass
import concourse.mybir as mybir
from concourse import bacc
from concourse.tile import TileContext
from concourse.bass_utils import run_bass_kernel_spmd
from concourse.masks import make_identity

FP = mybir.dt.float32
FPR = mybir.dt.float32r
BF = mybir.dt.bfloat16

N = 2048          # sequence length
CD = 1024         # model dim
HD = 512          # head-dim cols per core (8 heads x 64)
D = 64            # dim per head
NHEAD = 8         # heads per core
SCALE = 0.125     # D ** -0.5
NMT = HD // 128   # 4 head pairs (2 heads per 128-partition tile)
NCC = CD // 128   # 8 contraction chunks
NKB = N // 128    # 16 key blocks of 128
QC = 1024         # query chunk width
NQC = N // QC     # 2 chunks
VW = NKB * (D + 1) + D  # vsb flat width (1104): last U window needs 64 pad

DEBUG = False  # set True before _build_nc() to add intermediate DRAM dumps


def _spans(lo, hi):
    """Split [lo, hi) on the 512 grid: each matmul's PSUM output must stay
    within one 2KB bank (512 fp32 columns)."""
    out = []
    q0 = lo
    while q0 < hi:
        q1 = min(hi, (q0 // 512 + 1) * 512)
        out.append((q0, q1))
        q0 = q1
    return out


def _build_nc():
    nc = bacc.Bacc("TRN2", target_bir_lowering=False)
    x_d = nc.declare_dram_parameter("x", [N, CD], FP, isOutput=False)
    wq_d = nc.declare_dram_parameter("wq", [CD, HD], FP, isOutput=False)
    wk_d = nc.declare_dram_parameter("wk", [CD, HD], FP, isOutput=False)
    wv_d = nc.declare_dram_parameter("wv", [CD, HD], FP, isOutput=False)
    wo_d = nc.declare_dram_parameter("wo", [HD, CD], FP, isOutput=False)
    ctx_d = nc.declare_dram_parameter("ctx", [1, CD], FP, isOutput=False)
    y_d = nc.declare_dram_parameter("y", [N, CD], FP, isOutput=True)
    if DEBUG:
        dbg_q = nc.declare_dram_parameter("dbg_q", [128, QC], FP, isOutput=True)
        dbg_pcx = nc.declare_dram_parameter("dbg_pcx", [65, N], BF, isOutput=True)
        dbg_vsb = nc.declare_dram_parameter("dbg_vsb", [128, VW], BF, isOutput=True)
        dbg_u = nc.declare_dram_parameter("dbg_u", [2, 65, QC], FP, isOutput=True)
        dbg_at = nc.declare_dram_parameter("dbg_at", [128, N], BF, isOutput=True)
        dbg_pt = nc.declare_dram_parameter("dbg_pt", [128, QC], BF, isOutput=True)

    with TileContext(nc) as tc, \
         tc.tile_pool(name="pp", bufs=1) as pp, \
         tc.tile_pool(name="spp", bufs=2, space="PSUM") as sp_pool, \
         tc.tile_pool(name="pup", bufs=1, space="PSUM") as pu_pool, \
         tc.tile_pool(name="pjp", bufs=2, space="PSUM") as pj_pool, \
         tc.tile_pool(name="xnp", bufs=4) as xn_pool, \
         tc.tile_pool(name="wsp", bufs=2) as ws_pool, \
         tc.tile_pool(name="ptp", bufs=3) as pt_pool, \
         tc.tile_pool(name="usp", bufs=2) as us_pool, \
         tc.tile_pool(name="rcp", bufs=2) as rc_pool, \
         tc.tile_pool(name="ysp", bufs=3) as ys_pool:

        # ---- persistent SBUF tensors ----
        ident = pp.tile([128, 128], FP, tag="ident", name="ident")
        ident2 = pp.tile([128, 64], FP, tag="ident2", name="ident2")
        tri = pp.tile([128, 128], BF, tag="tri", name="tri")
        ones11 = pp.tile([1, 1], FP, tag="ones11", name="ones11")
        ones16 = pp.tile([128, 16], BF, tag="ones16", name="ones16")
        ctx_sb = pp.tile([1, CD], FP, tag="ctx_sb", name="ctx_sb")
        ctxT_sb = pp.tile([128, NCC], FPR, tag="ctxT_sb", name="ctxT_sb")
        zeros8 = pp.tile([128, NHEAD], FP, tag="zeros8", name="zeros8")
        identR = pp.tile([128, 128], FPR, tag="identR", name="identR")
        id2R = pp.tile([128, 64], FPR, tag="id2R", name="id2R")
        kctx_sb = pp.tile([1, HD], FP, tag="kctx_sb", name="kctx_sb")
        kct_sb = pp.tile([64, NHEAD], FPR, tag="kct_sb", name="kct_sb")
        kct2 = pp.tile([128, NHEAD], FPR, tag="kct2", name="kct2")
        # per-head v_ctx stationary padded to 128 cols so the ctx seed matmul
        # starts the full PSUM partition range the U matmuls accumulate into
        vctx_row = pp.tile([65, NHEAD * 128], BF, tag="vctx", name="vctx")
        # per (chunk, pair): q^T fp32r and attn^T bf16
        qkT = [[pp.tile([128, QC], FPR, tag=f"qkT{c}_{m}", name=f"qkT{c}_{m}")
                for m in range(NMT)] for c in range(NQC)]
        attnT = [[pp.tile([128, QC], BF, tag=f"at{c}_{m}", name=f"at{c}_{m}")
                  for m in range(NMT)] for c in range(NQC)]
        # ctx-score rows: heads of pair m at partitions 0 / 64
        pcx = [pp.tile([65, N], BF, tag=f"pcx{m}", name=f"pcx{m}") for m in range(NMT)]
        vsb = [pp.tile([128, VW], BF, tag=f"vsb{h}", name=f"vsb{h}") for h in range(NHEAD)]
        wq_sb = [pp.tile([128, HD], FPR, tag=f"wq{c}", name=f"wq{c}") for c in range(NCC)]
        wo_sb = [pp.tile([128, CD], BF, tag=f"wo{m}", name=f"wo{m}") for m in range(NMT)]
        xT_sb = [pp.tile([128, 512], FPR, tag=f"xT{c}", name=f"xT{c}") for c in range(NCC)]

        make_identity(nc, ident)
        nc.gpsimd.memset(ident2, 0.0)
        # ident2[p, f] = 1 where p == f or p == f + 64 (stacked 64x64 identities)
        nc.gpsimd.affine_select(
            out=ident2, in_=ident2, compare_op=mybir.AluOpType.not_equal,
            fill=1.0, base=0, pattern=[[-1, 64]], channel_multiplier=1)
        nc.gpsimd.affine_select(
            out=ident2, in_=ident2, compare_op=mybir.AluOpType.not_equal,
            fill=1.0, base=-64, pattern=[[-1, 64]], channel_multiplier=1)
        nc.vector.memset(ones11, 1.0)
        nc.vector.memset(ones16, 1.0)
        nc.vector.memset(zeros8, 0.0)
        nc.vector.tensor_copy(identR, ident)
        nc.vector.tensor_copy(id2R, ident2)
        nc.vector.memset(tri, 1.0)
        # keep pt[key p, query f] where f >= p (causal within diagonal block)
        nc.gpsimd.affine_select(
            out=tri, in_=tri, compare_op=mybir.AluOpType.is_ge,
            fill=0.0, base=0, pattern=[[1, 128]], channel_multiplier=-1)
        for h in range(NHEAD):
            nc.gpsimd.memset(vsb[h], 0.0)

        # ---- weight / ctx DMA ----
        nc.sync.dma_start(ctx_sb, ctx_d[0:1, :])
        for c in range(NCC):
            nc.sync.dma_start(wq_sb[c], wq_d[128 * c:128 * (c + 1), :].bitcast(FPR))
        for m in range(NMT):
            wot = ws_pool.tile([128, CD], FP, tag="ws", name="wot")
            nc.sync.dma_start(wot, wo_d[128 * m:128 * (m + 1), :])
            nc.vector.tensor_copy(wo_sb[m], wot)

        # ---- context k/v projections (tiny) ----
        # ctx^T via K=1 matmuls: out[128,1] = ctx_chunk.T @ ones
        ctxT_ps = pj_pool.tile([128, 512], FP, tag="pj", name="ctxT_ps")
        for c in range(NCC):
            nc.tensor.matmul(ctxT_ps[:, c:c + 1], ctx_sb[0:1, 128 * c:128 * (c + 1)],
                             ones11, start=True, stop=True)
        nc.vector.tensor_copy(ctxT_sb, ctxT_ps[:, 0:NCC])

        # k_ctx / v_ctx rows: out[1, 512] = ctx_chunk.T @ W chunk
        for wi, w_d in enumerate((wk_d, wv_d)):
            kv_ps = pj_pool.tile([128, 512], FP, tag="pj", name="kv_ps")
            for c in range(NCC):
                wt = ws_pool.tile([128, HD], FPR, tag="wskv", name="wt")
                nc.sync.dma_start(wt, w_d[128 * c:128 * (c + 1), :].bitcast(FPR))
                nc.tensor.matmul(kv_ps[0:1, :],
                                 ctxT_sb[:, c:c + 1], wt,
                                 start=(c == 0), stop=(c == NCC - 1))
            if wi == 0:
                nc.vector.tensor_copy(kctx_sb, kv_ps[0:1, :])
            else:
                nc.vector.memset(vctx_row[0:1, :], 0.0)
                vr3 = vctx_row.rearrange("p (h e) -> p h e", e=128)
                nc.vector.tensor_copy(
                    vr3[0:1, :, 0:D],
                    kv_ps[0:1, 0:NHEAD * D].rearrange("p (h e) -> p h e", e=D))
                nc.vector.tensor_copy(vr3[0:1, :, D:D + 1], ones16[0:1, 0:NHEAD])
                nc.sync.dma_start(vctx_row[64:65, :], vctx_row[0:1, :])

        # k_ctx^T per head; odd heads shifted to the 64-partition band
        kct_ps = pj_pool.tile([128, 512], FP, tag="pj", name="kct_ps")
        for h in range(NHEAD):
            nc.tensor.transpose(kct_ps[0:64, h:h + 1],
                                kctx_sb[0:1, 64 * h:64 * h + 64], ones11)
        nc.vector.tensor_copy(kct_sb, kct_ps[0:64, 0:NHEAD])
        nc.vector.tensor_copy(kct2, zeros8)
        for h in range(NHEAD):
            if h % 2 == 0:
                nc.vector.tensor_copy(kct2[0:64, h:h + 1], kct_sb[:, h:h + 1])
            else:
                nc.sync.dma_start(kct2[64:128, h:h + 1], kct_sb[:, h:h + 1])

        # ---- main per-chunk pipeline ----
        for c in range(NQC):
            lo, hi = QC * c, QC * (c + 1)
            kmax = (hi // 128)

            # projection: x^T then q^T for this chunk's 1024 queries
            for s in range(QC // 512):
                blk = lo + 512 * s
                xts = []
                for r in range(4):
                    xt = xn_pool.tile([128, CD], FPR, tag="xn", name="xn")
                    nc.sync.dma_start(xt, x_d[blk + 128 * r: blk + 128 * (r + 1), :].bitcast(FPR))
                    xts.append(xt)
                for c8 in range(NCC):
                    tps = pj_pool.tile([128, 512], FPR, tag="pj", name="tps")
                    for r in range(4):
                        nc.tensor.transpose(
                            tps[:, 128 * r:128 * (r + 1)],
                            xts[r][:, 128 * c8:128 * (c8 + 1)],
                            identR)
                    nc.vector.tensor_copy(xT_sb[c8], tps)
                for m in range(NMT):
                    qps = pj_pool.tile([128, 512], FP, tag="pj", name="qps")
                    for c8 in range(NCC):
                        nc.tensor.matmul(qps,
                                         wq_sb[c8][:, 128 * m:128 * (m + 1)],
                                         xT_sb[c8],
                                         start=(c8 == 0), stop=(c8 == NCC - 1))
                    nc.vector.tensor_copy(qkT[c][m][:, 512 * s:512 * (s + 1)], qps)

            # ctx score rows for this chunk: one matmul + one exp covers both
            # heads of the pair (kct2 is zero-banded per head, so the full-K
            # contraction against the pair's stacked q^T is exact); the odd
            # head's row is then DMA-shifted to partition 64 for the seeds
            for m in range(NMT):
                pps = sp_pool.tile([128, QC], FP, tag="sp", name="pps")
                for s2 in range(2):
                    nc.tensor.matmul(pps[0:2, 512 * s2:512 * (s2 + 1)],
                                     kct2[:, 2 * m:2 * m + 2],
                                     qkT[c][m][:, 512 * s2:512 * (s2 + 1)],
                                     start=True, stop=True)
                nc.scalar.activation(pcx[m][0:2, lo:hi], pps[0:2, :],
                                     mybir.ActivationFunctionType.Exp, scale=SCALE)
                nc.sync.dma_start(pcx[m][64:65, lo:hi], pcx[m][1:2, lo:hi])
            if DEBUG:
                (nc.sync.dma_start(dbg_q[:, :], qkT[0][0].bitcast(FP)) if c == 0 else None)
                nc.sync.dma_start(dbg_pcx[:, lo:hi], pcx[0][:, lo:hi])

            # v-blocks for this chunk's keys (all heads)
            for h in range(NHEAD):
                m, band = h // 2, 64 * (h % 2)
                vs3 = vsb[h][:, 0:NKB * (D + 1)].rearrange("p (a b) -> p a b", b=D + 1)
                vt = pj_pool.tile([128, 512], FPR, tag="pj", name="vt")
                for j in range(8):
                    nc.tensor.transpose(
                        vt[:, 64 * j:64 * (j + 1)],
                        qkT[c][m][band:band + 64, 128 * j:128 * j + 128],
                        id2R[band:band + 64, 0:64])
                nc.vector.tensor_copy(
                    vs3[:, 8 * c:8 * c + 8, 0:D],
                    vt.rearrange("p (j e) -> p j e", e=D))
                nc.vector.tensor_copy(vs3[:, 8 * c:8 * c + 8, D:D + 1], ones16[:, 0:8])
                if DEBUG and h == 0 and c == NQC - 1:
                    nc.sync.dma_start(dbg_vsb[:, :], vsb[0])

            # attention for this chunk, heads pipelined
            for h in range(NHEAD):
                m, band = h // 2, 64 * (h % 2)
                pu = pu_pool.tile([128, QC], FP, tag="pu", name="pu")
                # ctx contribution seeds the accumulator
                for s2 in range(2):
                    nc.tensor.matmul(pu[:, 512 * s2:512 * (s2 + 1)],
                                     vctx_row[band:band + 1, 128 * h:128 * h + 128],
                                     pcx[m][band:band + 1, lo + 512 * s2:lo + 512 * (s2 + 1)],
                                     start=True, stop=False)

                sp_tiles = {}

                def emit_S(kb, m=m, band=band, c=c, lo=lo, hi=hi, sp_tiles=sp_tiles):
                    i0 = 128 * (kb - 1)
                    clo = max(i0, lo)
                    spt = sp_pool.tile([128, QC], FP, tag="sp", name="spt")
                    kc, ko = i0 // QC, i0 % QC
                    keys = qkT[kc][m][band:band + 64, ko:ko + 128]
                    for (q0, q1) in _spans(clo, hi):
                        nc.tensor.matmul(spt[:, q0 - lo:q1 - lo],
                                         keys,
                                         qkT[c][m][band:band + 64, q0 - lo:q1 - lo],
                                         start=True, stop=True)
                    sp_tiles[kb] = (spt, clo)

                def emit_EU(kb, h=h, m=m, band=band, c=c, lo=lo, hi=hi,
                            kmax=kmax, pu=pu, sp_tiles=sp_tiles):
                    spt, clo = sp_tiles.pop(kb)
                    off = clo - lo
                    ptt = pt_pool.tile([128, QC], BF, tag="pt", name="ptt")
                    nc.scalar.activation(ptt[:, off:QC], spt[:, off:QC],
                                         mybir.ActivationFunctionType.Exp,
                                         scale=SCALE)
                    i0 = 128 * (kb - 1)
                    if i0 >= lo:
                        nc.vector.tensor_mul(ptt[:, off:off + 128],
                                             ptt[:, off:off + 128], tri)
                    if DEBUG and h == 0 and c == 0 and kb == 1:
                        nc.sync.dma_start(dbg_pt[:, :], ptt)
                    for (q0, q1) in _spans(clo, hi):
                        # each 512-col PSUM bank closes when its last-touching
                        # key block writes it (later blocks only cover q >= i0)
                        nc.tensor.matmul(pu[:, q0 - lo:q1 - lo],
                                         vsb[h][:, 65 * (kb - 1):65 * (kb - 1) + 128],
                                         ptt[:, q0 - lo:q1 - lo],
                                         start=False,
                                         stop=(kb == min(kmax, q1 // 128)))

                emit_S(1)
                for kb in range(1, kmax + 1):
                    if kb + 1 <= kmax:
                        emit_S(kb + 1)
                    emit_EU(kb)

                # normalize: attnT = U[0:64] / U[64]; copy U out of PSUM first
                # so the pu slot frees fast. The denominator row is copied to
                # partition 0 — reciprocal_approx_fast misreads partition-64
                # sources on hardware.
                u_sb = us_pool.tile([64, QC], FP, tag="us", name="u_sb")
                nc.vector.tensor_copy(u_sb, pu[0:64, :])
                den0 = rc_pool.tile([1, QC], FP, tag="rc0", name="den0")
                nc.vector.tensor_copy(den0, pu[64:65, :])
                if DEBUG and h == 0:
                    nc.sync.dma_start(dbg_u[c, 0:64, :], u_sb)
                    nc.sync.dma_start(dbg_u[c, 64:65, :], den0)
                r1 = rc_pool.tile([1, QC], FP, tag="rc1", name="r1")
                nc.vector.reciprocal_approx_fast(r1, den0)
                rb = rc_pool.tile([64, QC], FP, tag="rcb", name="rb")
                nc.gpsimd.partition_broadcast(rb, r1)
                nc.vector.tensor_mul(attnT[c][m][band:band + 64, :],
                                     u_sb, rb)
                if DEBUG and h == 1:
                    nc.sync.dma_start(dbg_at[:, lo:hi], attnT[c][0])

        # ---- output projection (chunk 0 then chunk 1; fills PE gaps during
        # attention of chunk 1 via the list scheduler) ----
        for c in range(NQC):
            for nb in range(QC // 128):
                for co in range(2):
                    py = pj_pool.tile([128, 512], FP, tag="pj", name="py")
                    for m in range(NMT):
                        nc.tensor.matmul(py,
                                         attnT[c][m][:, 128 * nb:128 * (nb + 1)],
                                         wo_sb[m][:, 512 * co:512 * (co + 1)],
                                         start=(m == 0), stop=(m == NMT - 1))
                    ysb = ys_pool.tile([128, 512], FP, tag="ysb", name="ysb")
                    nc.vector.tensor_copy(ysb, py)
                    nc.sync.dma_start(
                        y_d[QC * c + 128 * nb:QC * c + 128 * (nb + 1),
                            512 * co:512 * (co + 1)],
                        ysb)

    nc.compile()
    return nc


_NC = None


def _get_nc():
    global _NC
    if _NC is None:
        _NC = _build_nc()
    return _NC


def _shard(inputs):
    x = np.ascontiguousarray(np.asarray(inputs["x"], dtype=np.float32))
    context = np.ascontiguousarray(np.asarray(inputs["context"], dtype=np.float32))
    Wq = np.asarray(inputs["Wq"], dtype=np.float32)
    Wk = np.asarray(inputs["Wk"], dtype=np.float32)
    Wv = np.asarray(inputs["Wv"], dtype=np.float32)
    Wo = np.asarray(inputs["Wo"], dtype=np.float32)
    in_maps = []
    for c in range(8):
        b, g = c // 2, c % 2
        sl = slice(HD * g, HD * (g + 1))
        in_maps.append({
            "x": np.ascontiguousarray(x[b]),
            "wq": np.ascontiguousarray(Wq[:, sl]),
            "wk": np.ascontiguousarray(Wk[:, sl]),
            "wv": np.ascontiguousarray(Wv[:, sl]),
            "wo": np.ascontiguousarray(Wo[sl, :]),
            "ctx": np.ascontiguousarray(context[b:b + 1]),
        })
    return in_maps


def _run(inputs, trace=False, **kw):
    nc = _get_nc()
    in_maps = _shard(inputs)
    res = run_bass_kernel_spmd(nc, in_maps, list(range(8)), trace=trace, **kw)
    bo = np.asarray(inputs["bo"], dtype=np.float32)
    B = np.asarray(inputs["x"]).shape[0]
    y = np.empty((B, N, CD), dtype=np.float32)
    for b in range(B):
        y[b] = res.results[2 * b]["y"] + res.results[2 * b + 1]["y"] + bo
    return y, res


def kernel(**inputs):
    y, _ = _run(inputs)
    return y
